# revision 38
# baseline (speedup 1.0000x reference)
"""Trainium2 Bass kernel for nn_MHSA_40346922778634.

Math (per batch b, head h; the reference computes-then-drops the register
group, so reg_qk/reg_v are dead inputs):
  X = x[b] as [C=512, N=1024]
  Q = Wq X + bq ; K = Wk X + bk ; V = Wv X + bv       (per head: [64, N])
  P_h = (rel_h + rel_w) reshaped [head, 64, N]
  E[n,m] = Q_h[:,n].K_h[:,m] + P_h[:,n].Q_h[:,m]      ([N, N])
  attn = softmax(E, axis=-1)  (over m)
  out[b, h*64:(h+1)*64] = V_h @ attn^T + X[h*64:(h+1)*64]

Kernel strategy (8 cores, data-parallel over batch, 2 batches/core):
  - Z-projection with interleaved weights Wz = [Wk_h; Wq_h] per head chunk
    produces Z_h = [K_h; Q_h] stacked on 128 partitions directly (no
    partition-shift copies).  U_h = [Q_h; P_h]: pos rows preloaded once into
    partitions 64-127, Q rows copied per head with one SBUF->SBUF DMA.
  - E^T = Z_h^T U_h, one K=128 matmul pass per 128-row chunk (fp16).
  - exp without max-subtraction (logits bounded, fp32 PSUM), T = exp(E^T)
    stored bf16 (needs bf16 range).
  - AV with ones-augmented V^T (65 cols per head, 65th = 1.0 -> denominator
    in row 64), bf16.  AV of head h-1 interleaved with energy of head h at
    j-chunk granularity to keep PE dense.
  - Unnormalized O staged to SBUF bf16; per batch ONE [8, N]
    reciprocal_approx_fast (DVE reciprocal cost scales with free-dim length
    only, so batching heads on partitions is 8x cheaper than per-head
    [64, N]), hi+lo bf16 split, per-head rank-1 broadcast matmul, DVE
    multiply, GpSimd residual add (fp16 x), fp16 store.  The normalize tail
    of batch b overlaps batch b+1 compute.
"""

import sys

import numpy as np

try:
    import concourse.bass as bass  # noqa: F401
except Exception:  # pragma: no cover
    sys.path.insert(0, "/opt/trn_rl_repo")

import ml_dtypes
import concourse.bass as bass  # noqa: F401
import concourse.tile as tile
from concourse import bacc, mybir
from concourse.bass_utils import run_bass_kernel_spmd

F32 = mybir.dt.float32
F16 = mybir.dt.float16
BF16 = mybir.dt.bfloat16
EXP = mybir.ActivationFunctionType.Exp

N_CORES = 8
B, C, WD, HD = 16, 512, 32, 32
HEAD, D, N = 8, 64, 1024
BPC = B // N_CORES  # batches per core


def build_bass():
    nc = bacc.Bacc("TRN2")

    xh_d = nc.dram_tensor("xh", [BPC, C, N], F16, kind="ExternalInput")
    wzta_d = nc.dram_tensor("wzta", [4, 128, 256], F16, kind="ExternalInput")
    wztb_d = nc.dram_tensor("wztb", [4, 128, 768], F16, kind="ExternalInput")
    bz_d = nc.dram_tensor("bz", [128, 8], F32, kind="ExternalInput")
    wvpt_d = nc.dram_tensor("wvpt", [4, 128, 520], F16, kind="ExternalInput")
    bvp_d = nc.dram_tensor("bvp", [1, 520], F16, kind="ExternalInput")
    pos_d = nc.dram_tensor("pos", [HEAD, D, N], F16, kind="ExternalInput")
    mask_d = nc.dram_tensor("mask", [8, 512], BF16, kind="ExternalInput")
    out_d = nc.dram_tensor("out", [BPC, C, N], F16, kind="ExternalOutput")

    with tile.TileContext(nc) as tc:
        with (
            tc.tile_pool(name="consts", bufs=1) as cpool,
            tc.tile_pool(name="work", bufs=2) as wpool,
            tc.tile_pool(name="psume", bufs=2, space="PSUM") as pse,
            tc.tile_pool(name="psumo", bufs=4, space="PSUM") as pso,
        ):
            # ---- batch-0 X first (unblocks first projection ASAP) ----
            # per-kc tiles keep dependency tracking granular: the first
            # projection matmul only waits for x chunk 0 + wzt chunk 0.
            def prep_x(b):
                x_sb = [
                    wpool.tile([128, N], F16, name=f"x_{b}_{kc}", tag=f"x{kc}")
                    for kc in range(4)
                ]
                for kc in range(4):
                    nc.sync.dma_start(x_sb[kc][:], xh_d[b, kc * 128:(kc + 1) * 128, :])
                return x_sb

            def prep_xodd(b):
                # odd heads' residual rows live at partitions 64-127 of x;
                # engines need matching base partitions, so shift them to 0.
                cx = ctx[b]
                xodd = wpool.tile([64, 4, N], F16, name=f"xodd_{b}", tag="xodd")
                for kc in range(4):
                    nc.sync.dma_start(xodd[:, kc, :], cx["x"][kc][64:128, :])
                cx["xodd"] = xodd

            ctx = {0: {}}
            # DMA queue order == emission order and all active queues share
            # HBM bandwidth round-robin, so enqueue ONLY the bytes gating the
            # first two zproj chunks first (wzt cols 0:256 + x + bias + pos0).
            wztA = [
                cpool.tile([128, 256], F16, name=f"wztA{kc}") for kc in range(4)
            ]
            wztB = [
                cpool.tile([128, 768], F16, name=f"wztB{kc}") for kc in range(4)
            ]
            bz_sb = cpool.tile([128, 8], F32, name="bz_sb")
            # alternate the two HWDGE dispatch engines (sync/scalar) so the
            # ~0.6us-per-dma_start dispatch cost doesn't serialize the
            # critical prologue loads
            x_sb0 = [
                wpool.tile([128, N], F16, name=f"x_0_{kc}", tag=f"x{kc}")
                for kc in range(4)
            ]
            for kc in range(4):
                nc.sync.dma_start(x_sb0[kc][:], xh_d[0, kc * 128:(kc + 1) * 128, :])
                nc.scalar.dma_start(wztA[kc][:], wzta_d[kc])
            nc.scalar.dma_start(bz_sb[:], bz_d[:])
            ctx[0]["x"] = x_sb0
            # U tiles: partitions 64-127 = pos (loaded once, reused across
            # batches), partitions 0-63 = Q_h (DMA'd per batch per head).
            uall = wpool.tile([128, 8, N], F16, name="uall", tag="uall", bufs=1)
            nc.sync.dma_start(uall[64:128, 0, :], pos_d[0])
            for kc in range(4):
                nc.sync.dma_start(wztB[kc][:], wztb_d[kc])
            for h in range(1, HEAD):
                nc.sync.dma_start(uall[64:128, h, :], pos_d[h])
            wvpt_sb = cpool.tile([128, 4, 520], F16, name="wvpt_sb")
            for kc in range(4):
                nc.sync.dma_start(wvpt_sb[:, kc, :], wvpt_d[kc])
            bvp_sb = cpool.tile([1, 520], F16, name="bvp_sb")
            nc.sync.dma_start(bvp_sb[:], bvp_d[:])
            mask_sb = cpool.tile([8, 512], BF16, name="mask_sb")
            nc.sync.dma_start(mask_sb[:], mask_d[:])
            prep_xodd(0)
            ones1 = cpool.tile([1, 128], F16, name="ones1")
            nc.vector.memset(ones1[:], 1.0)
            zbias = cpool.tile([128, 1], F32, name="zbias")
            nc.vector.memset(zbias[:], 0.0)
            # normalize-tail tiles (shared across batches; memset so rank-1
            # broadcast matmuls never read uninitialized bits as NaN)
            rinv = wpool.tile([8, N], F32, name="rinv", tag="rinv", bufs=1)
            hi_t = wpool.tile([8, N], BF16, name="hi_t", tag="hi", bufs=1)
            nc.vector.memset(hi_t[:], 0.0)

            def emit_zproj_chunk(b, h):
                # Z_h = [K_h; Q_h] directly from interleaved weights; then
                # prefetch U_h's Q rows with one SBUF->SBUF DMA.
                cx = ctx[b]
                if "zall" not in cx:
                    cx["zall"] = wpool.tile(
                        [128, 8, N], F16, name=f"zall_{b}", tag="zall", bufs=2
                    )
                zall = cx["zall"]
                for nh in range(2):
                    ps = pso.tile([128, 512], F32, name=f"ps_z{b}{h}{nh}", tag="pso")
                    for kc in range(4):
                        if h < 2:
                            wslice = wztA[kc][:, h * 128:(h + 1) * 128]
                        else:
                            wslice = wztB[kc][:, (h - 2) * 128:(h - 1) * 128]
                        nc.tensor.matmul(
                            ps[:],
                            wslice,
                            cx["x"][kc][:, nh * 512:(nh + 1) * 512],
                            start=(kc == 0),
                            stop=(kc == 3),
                        )
                    nc.vector.tensor_scalar_add(
                        zall[:, h, nh * 512:(nh + 1) * 512], ps[:], bz_sb[:, h:h + 1]
                    )
                nc.sync.dma_start(uall[0:64, h, :], zall[64:128, h, :])

            def get_vpt(b):
                cx = ctx[b]
                if "vpt" not in cx:
                    cx["vpt"] = wpool.tile([128, 8, 520], BF16, name=f"vpt_{b}", tag="vpt")
                return cx["vpt"]

            def emit_vproj(b, c0, c1):
                # V^T padded projection (bf16 out), ones-column included via
                # the padded bias row; main/tail split keeps PSUM slots 1-bank.
                cx = ctx[b]
                vpt = get_vpt(b)
                for nc8 in range(c0, c1):
                    for (lo, hi) in ((0, 512), (512, 520)):
                        ps = pso.tile(
                            [128, hi - lo], F32, name=f"ps_v{b}{nc8}{lo}", tag="pso"
                        )
                        for kc in range(4):
                            nc.tensor.matmul(
                                ps[:],
                                cx["x"][kc][:, nc8 * 128:(nc8 + 1) * 128],
                                wvpt_sb[:, kc, lo:hi],
                                start=(kc == 0),
                                stop=False,
                            )
                        nc.tensor.matmul(
                            ps[:],
                            ones1[0:1, :],
                            bvp_sb[:, lo:hi],
                            start=False,
                            stop=True,
                        )
                        nc.vector.tensor_copy(vpt[:, nc8, lo:hi], ps[:])

            def get_osb(b):
                cx = ctx[b]
                if "osb" not in cx:
                    cx["osb"] = wpool.tile(
                        [65, 8, N], BF16, name=f"osb_{b}", tag="osb", bufs=2
                    )
                    cx["den_bf"] = wpool.tile(
                        [8, N], BF16, name=f"denbf_{b}", tag="denbf", bufs=2
                    )
                    cx["denf"] = wpool.tile(
                        [8, N], F32, name=f"denf_{b}", tag="denf", bufs=2
                    )
                    # rows are recip'd before all 8 heads land: keep unwritten
                    # rows finite (recip(1)=1) so masked-out lanes never NaN
                    nc.vector.memset(cx["den_bf"][:], 1.0)
                return cx["osb"], cx["den_bf"], cx["denf"]

            def emit_av_chunk(st, j):
                bp, hp, ptts, ops_a, ops_b = st
                pvpt = get_vpt(bp)
                for mh, ops in ((0, ops_a), (1, ops_b)):
                    nc.tensor.matmul(
                        ops[:],
                        pvpt[:, j, hp * 65:hp * 65 + 65],
                        ptts[j][:, mh * 512:(mh + 1) * 512],
                        start=(j == 0),
                        stop=(j == 7),
                    )

            def emit_av_evac(st, on_act=False):
                # evac normally on DVE (ACT-side evacs delay the next window's
                # first exps, stalling the lag-2 AV chain); the final head's
                # evac uses the by-then-idle Scalar engine.
                bp, hp, ptts, ops_a, ops_b = st
                osb, den_bf, denf = get_osb(bp)
                eng = nc.scalar.copy if on_act else nc.vector.tensor_copy
                eng(osb[:, hp, 0:512], ops_a[:])
                eng(osb[:, hp, 512:1024], ops_b[:])
                nc.sync.dma_start(den_bf[hp:hp + 1, :], osb[64:65, hp, :])

            def emit_head(b, h, carry, self_av=True, lag=3):
                # energy+exp for (b, h) with THIS head's AV interleaved at
                # lag-1 chunks (exp j-1 is done while E j streams).  The
                # previous head's final AV chunk + evac land at j=0 so its
                # normalize chain starts a full window earlier.
                cx = ctx[b]
                zall = cx["zall"]
                tts = []
                st = None
                for j in range(8):
                    eps = pse.tile([128, N], F32, name=f"ps_e{b}{h}{j}", tag="pse")
                    for ih in range(2):
                        nc.tensor.matmul(
                            eps[:, ih * 512:(ih + 1) * 512],
                            zall[:, h, j * 128:(j + 1) * 128],
                            uall[:, h, ih * 512:(ih + 1) * 512],
                            start=True,
                            stop=True,
                        )
                    if j in (2, 3, 4) and carry is not None:
                        emit_av_chunk(carry, j + 3)
                    if j == 5 and carry is not None:
                        emit_av_evac(carry)
                    if j > lag - 1 and self_av:
                        if st is None:
                            oa = pso.tile([65, 512], F32, name=f"ps_oa{b}{h}", tag="pso")
                            ob = pso.tile([65, 512], F32, name=f"ps_ob{b}{h}", tag="pso")
                            st = (b, h, tts, oa, ob)
                        emit_av_chunk(st, j - lag)
                    tt = wpool.tile([128, N], BF16, name=f"tt_{b}_{h}_{j}", tag="tt", bufs=12)
                    nc.scalar.activation(tt[:], eps[:], EXP, bias=zbias[:])
                    tts.append(tt)
                if not self_av:
                    return (b, h, tts, None, None)
                return st

            def emit_norm(b, h0, h1, use_pe=False):
                # normalize heads [h0, h1): reciprocal on just those den rows
                # (cost scales with free length, not partitions), hi+lo bf16
                # split, rank-1 broadcast, multiply, residual add, store.
                cx = ctx[b]
                osb, den_bf, denf = cx["osb"], cx["den_bf"], cx["denf"]
                # custom-DVE ops must start at partition 0: always process all
                # 8 rows (cost scales with free length, not partition count)
                nc.vector.tensor_copy(denf[:], den_bf[:])
                nc.vector.reciprocal_approx_fast(rinv[:], denf[:])
                nc.vector.tensor_copy(hi_t[:], rinv[:])
                for h in range(h0, h1):
                    nmul = wpool.tile([64, N], F16, name=f"nm_{b}_{h}", tag="nm", bufs=3)
                    if use_pe:
                        # low-latency PE rank-1 broadcast for tail-critical heads
                        for mh in range(2):
                            rb = pso.tile([64, 512], F32, name=f"ps_r{b}{h}{mh}", tag="pso")
                            nc.tensor.matmul(
                                rb[:], mask_sb[:, h * 64:(h + 1) * 64],
                                hi_t[:, mh * 512:(mh + 1) * 512],
                                start=True, stop=True,
                            )
                            nc.vector.tensor_mul(
                                nmul[:, mh * 512:(mh + 1) * 512],
                                osb[0:64, h, mh * 512:(mh + 1) * 512], rb[:],
                            )
                    else:
                        # SWDGE partition-broadcast (source must sit at
                        # partition 0, so hop the row there first) keeps the
                        # rank-1 replicate off the PE and makes the multiply
                        # a 16-bit 2x-mode DVE op
                        hst = wpool.tile([1, N], BF16, name=f"hst_{b}_{h}", tag="hst", bufs=2)
                        nc.sync.dma_start(hst[:], hi_t[h:h + 1, :])
                        rbv = wpool.tile([64, N], BF16, name=f"rbv_{b}_{h}", tag="rbv", bufs=2)
                        nc.gpsimd.partition_broadcast(rbv[:], hst[0:1, :])
                        nc.vector.tensor_mul(nmul[:], osb[0:64, h, :], rbv[:])
                    fin = wpool.tile([64, N], F16, name=f"fin_{b}_{h}", tag="fin", bufs=3)
                    if h % 2 == 0:
                        xres = cx["x"][h // 2][0:64, :]
                    else:
                        xres = cx["xodd"][:, h // 2, :]
                    nc.vector.tensor_add(fin[:], nmul[:], xres)
                    nc.sync.dma_start(out_d[b, h * 64:(h + 1) * 64, :], fin[:])

            # ---- prologue: batch 0 projections, head 0 early ----
            # head 0's AV must be EMITTED after the vproj writes (Tile RAW
            # deps look backward in emission order), so defer it here.
            emit_zproj_chunk(0, 0)
            emit_zproj_chunk(0, 1)
            emit_zproj_chunk(0, 2)
            b0, h0, tts0, _, _ = emit_head(0, 0, None, self_av=False)
            for h in range(3, 8):
                emit_zproj_chunk(0, h)
            emit_vproj(0, 0, 8)
            oa = pso.tile([65, 512], F32, name="ps_oa00", tag="pso")
            ob = pso.tile([65, 512], F32, name="ps_ob00", tag="pso")
            carry = (b0, h0, tts0, oa, ob)
            for j in range(5):
                emit_av_chunk(carry, j)

            # ---- steady state ----
            for b in range(BPC):
                for h in range(8):
                    if b == 0 and h == 0:
                        continue  # emitted in prologue
                    prev = carry
                    last = (b == BPC - 1 and h == 7)
                    carry = emit_head(b, h, carry, lag=1 if last else 3)
                    if b > 0 and h in (0, 1, 2, 3):
                        emit_zproj_chunk(b, h + 4)
                    if prev is not None and prev[1] in (2, 4, 6, 7):
                        if prev[1] == 6 and prev[0] == BPC - 1:
                            pass  # deferred below the drain (tail priority)
                        else:
                            emit_norm(prev[0], *{2: (0, 3), 4: (3, 5),
                                                 6: (5, 7), 7: (7, 8)}[prev[1]])
                    if b + 1 < BPC:
                        if h == 2:
                            ctx[b + 1] = {"x": prep_x(b + 1)}
                            prep_xodd(b + 1)
                        elif h in (4, 5, 6, 7):
                            emit_zproj_chunk(b + 1, h - 4)
                        if h == 6:
                            emit_vproj(b + 1, 0, 4)
                        elif h == 7:
                            emit_vproj(b + 1, 4, 8)

            # drain: last head's final AV chunks, then a half-pipelined
            # normalize chain (each 512-col half runs evac->den->recip->
            # broadcast->mul->add->store independently, halving tail latency)
            emit_av_chunk(carry, 7)
            bp, hp, ptts, ops_a, ops_b = carry
            osb, den_bf, denf = get_osb(bp)
            cxl = ctx[bp]
            nmul = wpool.tile([64, N], F16, name="nm_last", tag="nm", bufs=3)
            fin = wpool.tile([64, N], F16, name="fin_last", tag="fin", bufs=3)
            for mh, ops in ((0, ops_a), (1, ops_b)):
                sl = slice(mh * 512, (mh + 1) * 512)
                nc.scalar.copy(osb[:, hp, sl], ops[:])
                nc.sync.dma_start(den_bf[hp:hp + 1, sl], osb[64:65, hp, sl])
                nc.vector.tensor_copy(denf[:, sl], den_bf[:, sl])
                nc.vector.reciprocal_approx_fast(rinv[:, sl], denf[:, sl])
                nc.vector.tensor_copy(hi_t[:, sl], rinv[:, sl])
                rb = pso.tile([64, 512], F32, name=f"ps_rl{mh}", tag="pso")
                nc.tensor.matmul(
                    rb[:], mask_sb[:, hp * 64:(hp + 1) * 64], hi_t[:, sl],
                    start=True, stop=True,
                )
                nc.vector.tensor_mul(nmul[:, sl], osb[0:64, hp, sl], rb[:])
                nc.vector.tensor_add(
                    fin[:, sl], nmul[:, sl], cxl["xodd"][:, hp // 2, sl]
                )
                nc.sync.dma_start(out_d[bp, hp * 64:(hp + 1) * 64, sl], fin[:, sl])
            emit_norm(BPC - 1, 5, 7, use_pe=True)

    nc.compile()
    return nc


def _prep_consts(Wq, bq, Wk, bk, Wv, bv, rel_h, rel_w):
    # interleaved Z weights: chunk h rows 0-63 = Wk head h, rows 64-127 = Wq
    Wz = np.zeros((1024, 512), np.float32)
    bzv = np.zeros((1024,), np.float32)
    for h in range(HEAD):
        Wz[h * 128:h * 128 + 64] = Wk[h * 64:(h + 1) * 64]
        Wz[h * 128 + 64:h * 128 + 128] = Wq[h * 64:(h + 1) * 64]
        bzv[h * 128:h * 128 + 64] = bk[h * 64:(h + 1) * 64]
        bzv[h * 128 + 64:h * 128 + 128] = bq[h * 64:(h + 1) * 64]
    wzt = np.ascontiguousarray(Wz.T).reshape(4, 128, 1024).astype(np.float16)
    wzta = np.ascontiguousarray(wzt[:, :, 0:256])
    wztb = np.ascontiguousarray(wzt[:, :, 256:1024])
    bz = np.ascontiguousarray(bzv.reshape(8, 128).T).astype(np.float32)

    wvpt = np.zeros((512, 520), np.float32)
    bvp = np.zeros((1, 520), np.float32)
    for h in range(HEAD):
        wvpt[:, h * 65:h * 65 + 64] = Wv[h * 64:(h + 1) * 64, :].T
        bvp[0, h * 65:h * 65 + 64] = bv[h * 64:(h + 1) * 64]
        bvp[0, h * 65 + 64] = 1.0

    mask = np.zeros((8, 512), np.float32)
    for h in range(HEAD):
        mask[h, h * 64:(h + 1) * 64] = 1.0

    pos = (rel_h + rel_w).reshape(HEAD, D, N).astype(np.float16)
    return {
        "wzta": wzta,
        "wztb": wztb,
        "bz": bz,
        "wvpt": wvpt.reshape(4, 128, 520).astype(np.float16),
        "bvp": bvp.astype(np.float16),
        "mask": mask.astype(ml_dtypes.bfloat16),
        "pos": pos,
    }


_CACHE = {}


def build_in_maps(x, Wq, bq, Wk, bk, Wv, bv, rel_h, rel_w):
    x = np.asarray(x, np.float32)
    consts = _prep_consts(
        *[np.asarray(a, np.float32) for a in (Wq, bq, Wk, bk, Wv, bv, rel_h, rel_w)]
    )
    xh = x.reshape(B, C, N).astype(np.float16)
    in_maps = []
    for c in range(N_CORES):
        m = dict(consts)
        m["xh"] = np.ascontiguousarray(xh[c * BPC:(c + 1) * BPC])
        in_maps.append(m)
    return in_maps


def kernel(x, Wq, bq, Wk, bk, Wv, bv, rel_h, rel_w, reg_qk, reg_v):
    # reg_qk / reg_v are computed-then-dropped by the reference -> unused.
    in_maps = build_in_maps(x, Wq, bq, Wk, bk, Wv, bv, rel_h, rel_w)

    if "nc" not in _CACHE:
        _CACHE["nc"] = build_bass()
    res = run_bass_kernel_spmd(_CACHE["nc"], in_maps, list(range(N_CORES)))
    outs = [np.asarray(r["out"]).astype(np.float32) for r in res.results]
    return np.concatenate(outs, axis=0).reshape(B, C, WD, HD)


if __name__ == "__main__":
    nc = build_bass()
    print("built ok")


# revision 40
# speedup vs baseline: 1.0290x; 1.0290x over previous
"""Trainium2 Bass kernel for nn_MHSA_40346922778634.

Math (per batch b, head h; the reference computes-then-drops the register
group, so reg_qk/reg_v are dead inputs):
  X = x[b] as [C=512, N=1024]
  Q = Wq X + bq ; K = Wk X + bk ; V = Wv X + bv       (per head: [64, N])
  P_h = (rel_h + rel_w) reshaped [head, 64, N]
  E[n,m] = Q_h[:,n].K_h[:,m] + P_h[:,n].Q_h[:,m]      ([N, N])
  attn = softmax(E, axis=-1)  (over m)
  out[b, h*64:(h+1)*64] = V_h @ attn^T + X[h*64:(h+1)*64]

Kernel strategy (8 cores, data-parallel over batch, 2 batches/core):
  - Z-projection with interleaved weights Wz = [Wk_h; Wq_h] per head chunk
    produces Z_h = [K_h; Q_h] stacked on 128 partitions directly (no
    partition-shift copies).  U_h = [Q_h; P_h]: pos rows preloaded once into
    partitions 64-127, Q rows copied per head with one SBUF->SBUF DMA.
  - E^T = Z_h^T U_h, one K=128 matmul pass per 128-row chunk (fp16).
  - exp without max-subtraction (logits bounded, fp32 PSUM), T = exp(E^T)
    stored bf16 (needs bf16 range).
  - AV with ones-augmented V^T (65 cols per head, 65th = 1.0 -> denominator
    in row 64), bf16.  AV of head h-1 interleaved with energy of head h at
    j-chunk granularity to keep PE dense.
  - Unnormalized O staged to SBUF bf16; per batch ONE [8, N]
    reciprocal_approx_fast (DVE reciprocal cost scales with free-dim length
    only, so batching heads on partitions is 8x cheaper than per-head
    [64, N]), hi+lo bf16 split, per-head rank-1 broadcast matmul, DVE
    multiply, GpSimd residual add (fp16 x), fp16 store.  The normalize tail
    of batch b overlaps batch b+1 compute.
"""

import sys

import numpy as np

try:
    import concourse.bass as bass  # noqa: F401
except Exception:  # pragma: no cover
    sys.path.insert(0, "/opt/trn_rl_repo")

import ml_dtypes
import concourse.bass as bass  # noqa: F401
import concourse.tile as tile
from concourse import bacc, mybir
from concourse.bass_utils import run_bass_kernel_spmd

F32 = mybir.dt.float32
F16 = mybir.dt.float16
BF16 = mybir.dt.bfloat16
EXP = mybir.ActivationFunctionType.Exp

N_CORES = 8
B, C, WD, HD = 16, 512, 32, 32
HEAD, D, N = 8, 64, 1024
BPC = B // N_CORES  # batches per core


def build_bass():
    nc = bacc.Bacc("TRN2")

    xh_d = nc.dram_tensor("xh", [BPC, C, N], F16, kind="ExternalInput")
    wzta_d = nc.dram_tensor("wzta", [4, 128, 256], F16, kind="ExternalInput")
    wztb_d = nc.dram_tensor("wztb", [4, 128, 768], F16, kind="ExternalInput")
    bz_d = nc.dram_tensor("bz", [128, 8], F32, kind="ExternalInput")
    wvpt_d = nc.dram_tensor("wvpt", [4, 128, 520], F16, kind="ExternalInput")
    bvp_d = nc.dram_tensor("bvp", [1, 520], F16, kind="ExternalInput")
    pos_d = nc.dram_tensor("pos", [HEAD, D, N], F16, kind="ExternalInput")
    mask_d = nc.dram_tensor("mask", [8, 512], BF16, kind="ExternalInput")
    out_d = nc.dram_tensor("out", [BPC, C, N], F16, kind="ExternalOutput")

    with tile.TileContext(nc) as tc:
        with (
            tc.tile_pool(name="consts", bufs=1) as cpool,
            tc.tile_pool(name="work", bufs=2) as wpool,
            tc.tile_pool(name="psume", bufs=2, space="PSUM") as pse,
            tc.tile_pool(name="psumo", bufs=4, space="PSUM") as pso,
        ):
            # ---- batch-0 X first (unblocks first projection ASAP) ----
            # per-kc tiles keep dependency tracking granular: the first
            # projection matmul only waits for x chunk 0 + wzt chunk 0.
            def prep_x(b):
                x_sb = [
                    wpool.tile([128, N], F16, name=f"x_{b}_{kc}", tag=f"x{kc}")
                    for kc in range(4)
                ]
                for kc in range(4):
                    nc.sync.dma_start(x_sb[kc][:], xh_d[b, kc * 128:(kc + 1) * 128, :])
                return x_sb

            def prep_xodd(b):
                # odd heads' residual rows live at partitions 64-127 of x;
                # engines need matching base partitions, so shift them to 0.
                cx = ctx[b]
                xodd = wpool.tile([64, 4, N], F16, name=f"xodd_{b}", tag="xodd")
                for kc in range(4):
                    nc.sync.dma_start(xodd[:, kc, :], cx["x"][kc][64:128, :])
                cx["xodd"] = xodd

            ctx = {0: {}}
            # DMA queue order == emission order and all active queues share
            # HBM bandwidth round-robin, so enqueue ONLY the bytes gating the
            # first two zproj chunks first (wzt cols 0:256 + x + bias + pos0).
            wztA = [
                cpool.tile([128, 256], F16, name=f"wztA{kc}") for kc in range(4)
            ]
            wztB = [
                cpool.tile([128, 768], F16, name=f"wztB{kc}") for kc in range(4)
            ]
            bz_sb = cpool.tile([128, 8], F32, name="bz_sb")
            # alternate the two HWDGE dispatch engines (sync/scalar) so the
            # ~0.6us-per-dma_start dispatch cost doesn't serialize the
            # critical prologue loads
            x_sb0 = [
                wpool.tile([128, N], F16, name=f"x_0_{kc}", tag=f"x{kc}")
                for kc in range(4)
            ]
            for kc in range(4):
                nc.sync.dma_start(x_sb0[kc][:], xh_d[0, kc * 128:(kc + 1) * 128, :])
                nc.scalar.dma_start(wztA[kc][:], wzta_d[kc])
            nc.scalar.dma_start(bz_sb[:], bz_d[:])
            ctx[0]["x"] = x_sb0
            # U tiles: partitions 64-127 = pos (loaded once, reused across
            # batches), partitions 0-63 = Q_h (DMA'd per batch per head).
            uall = wpool.tile([128, 8, N], F16, name="uall", tag="uall", bufs=1)
            nc.sync.dma_start(uall[64:128, 0, :], pos_d[0])
            for kc in range(4):
                nc.sync.dma_start(wztB[kc][:], wztb_d[kc])
            for h in range(1, HEAD):
                nc.sync.dma_start(uall[64:128, h, :], pos_d[h])
            wvpt_sb = cpool.tile([128, 4, 520], F16, name="wvpt_sb")
            for kc in range(4):
                nc.sync.dma_start(wvpt_sb[:, kc, :], wvpt_d[kc])
            bvp_sb = cpool.tile([1, 520], F16, name="bvp_sb")
            nc.sync.dma_start(bvp_sb[:], bvp_d[:])
            mask_sb = cpool.tile([8, 512], BF16, name="mask_sb")
            nc.sync.dma_start(mask_sb[:], mask_d[:])
            prep_xodd(0)
            ones1 = cpool.tile([1, 128], F16, name="ones1")
            nc.vector.memset(ones1[:], 1.0)
            zbias = cpool.tile([128, 1], F32, name="zbias")
            nc.vector.memset(zbias[:], 0.0)
            # normalize-tail tiles (shared across batches; memset so rank-1
            # broadcast matmuls never read uninitialized bits as NaN)
            rinv = wpool.tile([8, N], F32, name="rinv", tag="rinv", bufs=1)
            hi_t = wpool.tile([8, N], BF16, name="hi_t", tag="hi", bufs=1)
            nc.vector.memset(hi_t[:], 0.0)

            def emit_zproj_chunk(b, h):
                # Z_h = [K_h; Q_h] directly from interleaved weights; then
                # prefetch U_h's Q rows with one SBUF->SBUF DMA.
                cx = ctx[b]
                if "zall" not in cx:
                    cx["zall"] = wpool.tile(
                        [128, 8, N], F16, name=f"zall_{b}", tag="zall", bufs=2
                    )
                zall = cx["zall"]
                for nh in range(2):
                    ps = pso.tile([128, 512], F32, name=f"ps_z{b}{h}{nh}", tag="pso")
                    for kc in range(4):
                        if h < 2:
                            wslice = wztA[kc][:, h * 128:(h + 1) * 128]
                        else:
                            wslice = wztB[kc][:, (h - 2) * 128:(h - 1) * 128]
                        nc.tensor.matmul(
                            ps[:],
                            wslice,
                            cx["x"][kc][:, nh * 512:(nh + 1) * 512],
                            start=(kc == 0),
                            stop=(kc == 3),
                        )
                    nc.vector.tensor_scalar_add(
                        zall[:, h, nh * 512:(nh + 1) * 512], ps[:], bz_sb[:, h:h + 1]
                    )
                nc.sync.dma_start(uall[0:64, h, :], zall[64:128, h, :])

            def get_vpt(b):
                cx = ctx[b]
                if "vpt" not in cx:
                    cx["vpt"] = wpool.tile([128, 8, 520], BF16, name=f"vpt_{b}", tag="vpt")
                return cx["vpt"]

            def emit_vproj(b, c0, c1):
                # V^T padded projection (bf16 out), ones-column included via
                # the padded bias row; main/tail split keeps PSUM slots 1-bank.
                cx = ctx[b]
                vpt = get_vpt(b)
                for nc8 in range(c0, c1):
                    for (lo, hi) in ((0, 512), (512, 520)):
                        ps = pso.tile(
                            [128, hi - lo], F32, name=f"ps_v{b}{nc8}{lo}", tag="pso"
                        )
                        for kc in range(4):
                            nc.tensor.matmul(
                                ps[:],
                                cx["x"][kc][:, nc8 * 128:(nc8 + 1) * 128],
                                wvpt_sb[:, kc, lo:hi],
                                start=(kc == 0),
                                stop=False,
                            )
                        nc.tensor.matmul(
                            ps[:],
                            ones1[0:1, :],
                            bvp_sb[:, lo:hi],
                            start=False,
                            stop=True,
                        )
                        nc.vector.tensor_copy(vpt[:, nc8, lo:hi], ps[:])

            def get_osb(b):
                cx = ctx[b]
                if "osb" not in cx:
                    cx["osb"] = wpool.tile(
                        [65, 8, N], BF16, name=f"osb_{b}", tag="osb", bufs=2
                    )
                    cx["den_bf"] = wpool.tile(
                        [8, N], BF16, name=f"denbf_{b}", tag="denbf", bufs=2
                    )
                    cx["denf"] = wpool.tile(
                        [8, N], F32, name=f"denf_{b}", tag="denf", bufs=2
                    )
                    # rows are recip'd before all 8 heads land: keep unwritten
                    # rows finite (recip(1)=1) so masked-out lanes never NaN
                    nc.vector.memset(cx["den_bf"][:], 1.0)
                return cx["osb"], cx["den_bf"], cx["denf"]

            def emit_av_chunk(st, j):
                bp, hp, ptts, ops_a, ops_b = st
                pvpt = get_vpt(bp)
                for mh, ops in ((0, ops_a), (1, ops_b)):
                    nc.tensor.matmul(
                        ops[:],
                        pvpt[:, j, hp * 65:hp * 65 + 65],
                        ptts[j][:, mh * 512:(mh + 1) * 512],
                        start=(j == 0),
                        stop=(j == 7),
                    )

            def emit_av_evac(st, on_act=False):
                # evac normally on DVE (ACT-side evacs delay the next window's
                # first exps, stalling the lag-2 AV chain); the final head's
                # evac uses the by-then-idle Scalar engine.
                bp, hp, ptts, ops_a, ops_b = st
                osb, den_bf, denf = get_osb(bp)
                eng = nc.scalar.copy if on_act else nc.vector.tensor_copy
                eng(osb[:, hp, 0:512], ops_a[:])
                eng(osb[:, hp, 512:1024], ops_b[:])
                nc.sync.dma_start(den_bf[hp:hp + 1, :], osb[64:65, hp, :])

            def emit_head(b, h, carry, self_av=True, lag=3):
                # energy+exp for (b, h) with THIS head's AV interleaved at
                # lag-1 chunks (exp j-1 is done while E j streams).  The
                # previous head's final AV chunk + evac land at j=0 so its
                # normalize chain starts a full window earlier.
                cx = ctx[b]
                zall = cx["zall"]
                tts = []
                st = None
                for j in range(8):
                    eps = pse.tile([128, N], F32, name=f"ps_e{b}{h}{j}", tag="pse")
                    for ih in range(2):
                        nc.tensor.matmul(
                            eps[:, ih * 512:(ih + 1) * 512],
                            zall[:, h, j * 128:(j + 1) * 128],
                            uall[:, h, ih * 512:(ih + 1) * 512],
                            start=True,
                            stop=True,
                        )
                    if j in (2, 3, 4) and carry is not None:
                        emit_av_chunk(carry, j + 3)
                    if j == 5 and carry is not None:
                        emit_av_evac(carry)
                    if j > lag - 1 and self_av:
                        if st is None:
                            oa = pso.tile([65, 512], F32, name=f"ps_oa{b}{h}", tag="pso")
                            ob = pso.tile([65, 512], F32, name=f"ps_ob{b}{h}", tag="pso")
                            st = (b, h, tts, oa, ob)
                        emit_av_chunk(st, j - lag)
                    tt = wpool.tile([128, N], BF16, name=f"tt_{b}_{h}_{j}", tag="tt", bufs=12)
                    nc.scalar.activation(tt[:], eps[:], EXP, bias=zbias[:])
                    tts.append(tt)
                if not self_av:
                    return (b, h, tts, None, None)
                return st

            def emit_norm(b, h0, h1, use_pe=False):
                # normalize heads [h0, h1): reciprocal on just those den rows
                # (cost scales with free length, not partitions), hi+lo bf16
                # split, rank-1 broadcast, multiply, residual add, store.
                cx = ctx[b]
                osb, den_bf, denf = cx["osb"], cx["den_bf"], cx["denf"]
                # custom-DVE ops must start at partition 0: always process all
                # 8 rows (cost scales with free length, not partition count)
                nc.vector.tensor_copy(denf[:], den_bf[:])
                nc.vector.reciprocal_approx_fast(rinv[:], denf[:])
                nc.vector.tensor_copy(hi_t[:], rinv[:])
                for h in range(h0, h1):
                    nmul = wpool.tile([64, N], F16, name=f"nm_{b}_{h}", tag="nm", bufs=3)
                    if use_pe:
                        # low-latency PE rank-1 broadcast for tail-critical heads
                        for mh in range(2):
                            rb = pso.tile([64, 512], F32, name=f"ps_r{b}{h}{mh}", tag="pso")
                            nc.tensor.matmul(
                                rb[:], mask_sb[:, h * 64:(h + 1) * 64],
                                hi_t[:, mh * 512:(mh + 1) * 512],
                                start=True, stop=True,
                            )
                            nc.vector.tensor_mul(
                                nmul[:, mh * 512:(mh + 1) * 512],
                                osb[0:64, h, mh * 512:(mh + 1) * 512], rb[:],
                            )
                    else:
                        # SWDGE partition-broadcast (source must sit at
                        # partition 0, so hop the row there first) keeps the
                        # rank-1 replicate off the PE and makes the multiply
                        # a 16-bit 2x-mode DVE op
                        hst = wpool.tile([1, N], BF16, name=f"hst_{b}_{h}", tag="hst", bufs=2)
                        nc.sync.dma_start(hst[:], hi_t[h:h + 1, :])
                        rbv = wpool.tile([64, N], BF16, name=f"rbv_{b}_{h}", tag="rbv", bufs=2)
                        nc.gpsimd.partition_broadcast(rbv[:], hst[0:1, :])
                        nc.vector.tensor_mul(nmul[:], osb[0:64, h, :], rbv[:])
                    fin = wpool.tile([64, N], F16, name=f"fin_{b}_{h}", tag="fin", bufs=3)
                    if h % 2 == 0:
                        xres = cx["x"][h // 2][0:64, :]
                    else:
                        xres = cx["xodd"][:, h // 2, :]
                    nc.vector.tensor_add(fin[:], nmul[:], xres)
                    nc.sync.dma_start(out_d[b, h * 64:(h + 1) * 64, :], fin[:])

            # ---- prologue: batch 0 projections, head 0 early ----
            # head 0's AV must be EMITTED after the vproj writes (Tile RAW
            # deps look backward in emission order), so defer it here.
            emit_zproj_chunk(0, 0)
            emit_zproj_chunk(0, 1)
            emit_zproj_chunk(0, 2)
            b0, h0, tts0, _, _ = emit_head(0, 0, None, self_av=False)
            for h in range(3, 8):
                emit_zproj_chunk(0, h)
            emit_vproj(0, 0, 8)
            oa = pso.tile([65, 512], F32, name="ps_oa00", tag="pso")
            ob = pso.tile([65, 512], F32, name="ps_ob00", tag="pso")
            carry = (b0, h0, tts0, oa, ob)
            for j in range(5):
                emit_av_chunk(carry, j)

            # ---- steady state ----
            for b in range(BPC):
                for h in range(8):
                    if b == 0 and h == 0:
                        continue  # emitted in prologue
                    prev = carry
                    last = (b == BPC - 1 and h == 7)
                    carry = emit_head(b, h, carry, lag=1 if last else 3)
                    if b > 0 and h in (0, 1, 2, 3):
                        emit_zproj_chunk(b, h + 4)
                    if prev is not None and prev[1] in (2, 4, 6, 7):
                        emit_norm(prev[0], *{2: (0, 3), 4: (3, 5),
                                             6: (5, 7), 7: (7, 8)}[prev[1]],
                                  use_pe=(prev[0] == BPC - 1 and prev[1] == 6))
                    if b + 1 < BPC:
                        if h == 2:
                            ctx[b + 1] = {"x": prep_x(b + 1)}
                            prep_xodd(b + 1)
                        elif h in (4, 5, 6, 7):
                            emit_zproj_chunk(b + 1, h - 4)
                        if h == 6:
                            emit_vproj(b + 1, 0, 4)
                        elif h == 7:
                            emit_vproj(b + 1, 4, 8)

            # drain: last head's final AV chunks, then a half-pipelined
            # normalize chain (each 512-col half runs evac->den->recip->
            # broadcast->mul->add->store independently, halving tail latency)
            emit_av_chunk(carry, 7)
            bp, hp, ptts, ops_a, ops_b = carry
            osb, den_bf, denf = get_osb(bp)
            cxl = ctx[bp]
            nmul = wpool.tile([64, N], F16, name="nm_last", tag="nm", bufs=3)
            fin = wpool.tile([64, N], F16, name="fin_last", tag="fin", bufs=3)
            for mh, ops in ((0, ops_a), (1, ops_b)):
                sl = slice(mh * 512, (mh + 1) * 512)
                nc.scalar.copy(osb[:, hp, sl], ops[:])
                nc.sync.dma_start(den_bf[hp:hp + 1, sl], osb[64:65, hp, sl])
                nc.vector.tensor_copy(denf[:, sl], den_bf[:, sl])
                nc.vector.reciprocal_approx_fast(rinv[:, sl], denf[:, sl])
                nc.vector.tensor_copy(hi_t[:, sl], rinv[:, sl])
                rb = pso.tile([64, 512], F32, name=f"ps_rl{mh}", tag="pso")
                nc.tensor.matmul(
                    rb[:], mask_sb[:, hp * 64:(hp + 1) * 64], hi_t[:, sl],
                    start=True, stop=True,
                )
                nc.vector.tensor_mul(nmul[:, sl], osb[0:64, hp, sl], rb[:])
                nc.vector.tensor_add(
                    fin[:, sl], nmul[:, sl], cxl["xodd"][:, hp // 2, sl]
                )
                nc.sync.dma_start(out_d[bp, hp * 64:(hp + 1) * 64, sl], fin[:, sl])

    nc.compile()
    return nc


def _prep_consts(Wq, bq, Wk, bk, Wv, bv, rel_h, rel_w):
    # interleaved Z weights: chunk h rows 0-63 = Wk head h, rows 64-127 = Wq
    Wz = np.zeros((1024, 512), np.float32)
    bzv = np.zeros((1024,), np.float32)
    for h in range(HEAD):
        Wz[h * 128:h * 128 + 64] = Wk[h * 64:(h + 1) * 64]
        Wz[h * 128 + 64:h * 128 + 128] = Wq[h * 64:(h + 1) * 64]
        bzv[h * 128:h * 128 + 64] = bk[h * 64:(h + 1) * 64]
        bzv[h * 128 + 64:h * 128 + 128] = bq[h * 64:(h + 1) * 64]
    wzt = np.ascontiguousarray(Wz.T).reshape(4, 128, 1024).astype(np.float16)
    wzta = np.ascontiguousarray(wzt[:, :, 0:256])
    wztb = np.ascontiguousarray(wzt[:, :, 256:1024])
    bz = np.ascontiguousarray(bzv.reshape(8, 128).T).astype(np.float32)

    wvpt = np.zeros((512, 520), np.float32)
    bvp = np.zeros((1, 520), np.float32)
    for h in range(HEAD):
        wvpt[:, h * 65:h * 65 + 64] = Wv[h * 64:(h + 1) * 64, :].T
        bvp[0, h * 65:h * 65 + 64] = bv[h * 64:(h + 1) * 64]
        bvp[0, h * 65 + 64] = 1.0

    mask = np.zeros((8, 512), np.float32)
    for h in range(HEAD):
        mask[h, h * 64:(h + 1) * 64] = 1.0

    pos = (rel_h + rel_w).reshape(HEAD, D, N).astype(np.float16)
    return {
        "wzta": wzta,
        "wztb": wztb,
        "bz": bz,
        "wvpt": wvpt.reshape(4, 128, 520).astype(np.float16),
        "bvp": bvp.astype(np.float16),
        "mask": mask.astype(ml_dtypes.bfloat16),
        "pos": pos,
    }


_CACHE = {}


def build_in_maps(x, Wq, bq, Wk, bk, Wv, bv, rel_h, rel_w):
    x = np.asarray(x, np.float32)
    consts = _prep_consts(
        *[np.asarray(a, np.float32) for a in (Wq, bq, Wk, bk, Wv, bv, rel_h, rel_w)]
    )
    xh = x.reshape(B, C, N).astype(np.float16)
    in_maps = []
    for c in range(N_CORES):
        m = dict(consts)
        m["xh"] = np.ascontiguousarray(xh[c * BPC:(c + 1) * BPC])
        in_maps.append(m)
    return in_maps


def kernel(x, Wq, bq, Wk, bk, Wv, bv, rel_h, rel_w, reg_qk, reg_v):
    # reg_qk / reg_v are computed-then-dropped by the reference -> unused.
    in_maps = build_in_maps(x, Wq, bq, Wk, bk, Wv, bv, rel_h, rel_w)

    if "nc" not in _CACHE:
        _CACHE["nc"] = build_bass()
    res = run_bass_kernel_spmd(_CACHE["nc"], in_maps, list(range(N_CORES)))
    outs = [np.asarray(r["out"]).astype(np.float32) for r in res.results]
    return np.concatenate(outs, axis=0).reshape(B, C, WD, HD)


if __name__ == "__main__":
    nc = build_bass()
    print("built ok")


# revision 41
# speedup vs baseline: 1.0352x; 1.0061x over previous
"""Trainium2 Bass kernel for nn_MHSA_40346922778634.

Math (per batch b, head h; the reference computes-then-drops the register
group, so reg_qk/reg_v are dead inputs):
  X = x[b] as [C=512, N=1024]
  Q = Wq X + bq ; K = Wk X + bk ; V = Wv X + bv       (per head: [64, N])
  P_h = (rel_h + rel_w) reshaped [head, 64, N]
  E[n,m] = Q_h[:,n].K_h[:,m] + P_h[:,n].Q_h[:,m]      ([N, N])
  attn = softmax(E, axis=-1)  (over m)
  out[b, h*64:(h+1)*64] = V_h @ attn^T + X[h*64:(h+1)*64]

Kernel strategy (8 cores, data-parallel over batch, 2 batches/core):
  - Z-projection with interleaved weights Wz = [Wk_h; Wq_h] per head chunk
    produces Z_h = [K_h; Q_h] stacked on 128 partitions directly (no
    partition-shift copies).  U_h = [Q_h; P_h]: pos rows preloaded once into
    partitions 64-127, Q rows copied per head with one SBUF->SBUF DMA.
  - E^T = Z_h^T U_h, one K=128 matmul pass per 128-row chunk (fp16).
  - exp without max-subtraction (logits bounded, fp32 PSUM), T = exp(E^T)
    stored bf16 (needs bf16 range).
  - AV with ones-augmented V^T (65 cols per head, 65th = 1.0 -> denominator
    in row 64), bf16.  AV of head h-1 interleaved with energy of head h at
    j-chunk granularity to keep PE dense.
  - Unnormalized O staged to SBUF bf16; per batch ONE [8, N]
    reciprocal_approx_fast (DVE reciprocal cost scales with free-dim length
    only, so batching heads on partitions is 8x cheaper than per-head
    [64, N]), hi+lo bf16 split, per-head rank-1 broadcast matmul, DVE
    multiply, GpSimd residual add (fp16 x), fp16 store.  The normalize tail
    of batch b overlaps batch b+1 compute.
"""

import sys

import numpy as np

try:
    import concourse.bass as bass  # noqa: F401
except Exception:  # pragma: no cover
    sys.path.insert(0, "/opt/trn_rl_repo")

import ml_dtypes
import concourse.bass as bass  # noqa: F401
import concourse.tile as tile
from concourse import bacc, mybir
from concourse.bass_utils import run_bass_kernel_spmd

F32 = mybir.dt.float32
F16 = mybir.dt.float16
BF16 = mybir.dt.bfloat16
EXP = mybir.ActivationFunctionType.Exp

N_CORES = 8
B, C, WD, HD = 16, 512, 32, 32
HEAD, D, N = 8, 64, 1024
BPC = B // N_CORES  # batches per core


def build_bass():
    nc = bacc.Bacc("TRN2")

    xh_d = nc.dram_tensor("xh", [BPC, C, N], F16, kind="ExternalInput")
    wzta_d = nc.dram_tensor("wzta", [4, 128, 256], F16, kind="ExternalInput")
    wztb_d = nc.dram_tensor("wztb", [4, 128, 768], F16, kind="ExternalInput")
    bz_d = nc.dram_tensor("bz", [128, 8], F32, kind="ExternalInput")
    wvpt_d = nc.dram_tensor("wvpt", [4, 128, 520], F16, kind="ExternalInput")
    bvp_d = nc.dram_tensor("bvp", [1, 520], F16, kind="ExternalInput")
    pos_d = nc.dram_tensor("pos", [HEAD, D, N], F16, kind="ExternalInput")
    mask_d = nc.dram_tensor("mask", [8, 512], BF16, kind="ExternalInput")
    out_d = nc.dram_tensor("out", [BPC, C, N], F16, kind="ExternalOutput")

    with tile.TileContext(nc) as tc:
        with (
            tc.tile_pool(name="consts", bufs=1) as cpool,
            tc.tile_pool(name="work", bufs=2) as wpool,
            tc.tile_pool(name="psume", bufs=2, space="PSUM") as pse,
            tc.tile_pool(name="psumo", bufs=4, space="PSUM") as pso,
        ):
            # ---- batch-0 X first (unblocks first projection ASAP) ----
            # per-kc tiles keep dependency tracking granular: the first
            # projection matmul only waits for x chunk 0 + wzt chunk 0.
            def prep_x(b):
                x_sb = [
                    wpool.tile([128, N], F16, name=f"x_{b}_{kc}", tag=f"x{kc}")
                    for kc in range(4)
                ]
                for kc in range(4):
                    nc.sync.dma_start(x_sb[kc][:], xh_d[b, kc * 128:(kc + 1) * 128, :])
                return x_sb

            def prep_xodd(b):
                # odd heads' residual rows live at partitions 64-127 of x;
                # engines need matching base partitions, so shift them to 0.
                cx = ctx[b]
                xodd = wpool.tile([64, 4, N], F16, name=f"xodd_{b}", tag="xodd")
                for kc in range(4):
                    nc.sync.dma_start(xodd[:, kc, :], cx["x"][kc][64:128, :])
                cx["xodd"] = xodd

            ctx = {0: {}}
            # HAM warmup: the PE clock sits gated at 1.2GHz until ~3.4us of
            # sustained activity.  Junk matmuls (deps: one memset only) run
            # during the input-DMA wait so real matmuls start at 2.4GHz.
            wrm = cpool.tile([1, 512], F16, name="wrm")
            nc.vector.memset(wrm[:], 0.0)
            for i in range(9):
                wps = pso.tile([128, 512], F32, name=f"ps_wrm{i}", tag="pso")
                nc.tensor.matmul(wps[:], wrm[0:1, 0:128], wrm[:], start=True, stop=True)
            # DMA queue order == emission order and all active queues share
            # HBM bandwidth round-robin, so enqueue ONLY the bytes gating the
            # first two zproj chunks first (wzt cols 0:256 + x + bias + pos0).
            wztA = [
                cpool.tile([128, 256], F16, name=f"wztA{kc}") for kc in range(4)
            ]
            wztB = [
                cpool.tile([128, 768], F16, name=f"wztB{kc}") for kc in range(4)
            ]
            bz_sb = cpool.tile([128, 8], F32, name="bz_sb")
            # alternate the two HWDGE dispatch engines (sync/scalar) so the
            # ~0.6us-per-dma_start dispatch cost doesn't serialize the
            # critical prologue loads
            x_sb0 = [
                wpool.tile([128, N], F16, name=f"x_0_{kc}", tag=f"x{kc}")
                for kc in range(4)
            ]
            for kc in range(4):
                nc.sync.dma_start(x_sb0[kc][:], xh_d[0, kc * 128:(kc + 1) * 128, :])
                nc.scalar.dma_start(wztA[kc][:], wzta_d[kc])
            nc.scalar.dma_start(bz_sb[:], bz_d[:])
            ctx[0]["x"] = x_sb0
            # U tiles: partitions 64-127 = pos (loaded once, reused across
            # batches), partitions 0-63 = Q_h (DMA'd per batch per head).
            uall = wpool.tile([128, 8, N], F16, name="uall", tag="uall", bufs=1)
            nc.sync.dma_start(uall[64:128, 0, :], pos_d[0])
            for kc in range(4):
                nc.sync.dma_start(wztB[kc][:], wztb_d[kc])
            for h in range(1, HEAD):
                nc.sync.dma_start(uall[64:128, h, :], pos_d[h])
            wvpt_sb = cpool.tile([128, 4, 520], F16, name="wvpt_sb")
            for kc in range(4):
                nc.sync.dma_start(wvpt_sb[:, kc, :], wvpt_d[kc])
            bvp_sb = cpool.tile([1, 520], F16, name="bvp_sb")
            nc.sync.dma_start(bvp_sb[:], bvp_d[:])
            mask_sb = cpool.tile([8, 512], BF16, name="mask_sb")
            nc.sync.dma_start(mask_sb[:], mask_d[:])
            prep_xodd(0)
            ones1 = cpool.tile([1, 128], F16, name="ones1")
            nc.vector.memset(ones1[:], 1.0)
            zbias = cpool.tile([128, 1], F32, name="zbias")
            nc.vector.memset(zbias[:], 0.0)
            # normalize-tail tiles (shared across batches; memset so rank-1
            # broadcast matmuls never read uninitialized bits as NaN)
            rinv = wpool.tile([8, N], F32, name="rinv", tag="rinv", bufs=1)
            hi_t = wpool.tile([8, N], BF16, name="hi_t", tag="hi", bufs=1)
            nc.vector.memset(hi_t[:], 0.0)

            def emit_zproj_chunk(b, h):
                # Z_h = [K_h; Q_h] directly from interleaved weights; then
                # prefetch U_h's Q rows with one SBUF->SBUF DMA.
                cx = ctx[b]
                if "zall" not in cx:
                    cx["zall"] = wpool.tile(
                        [128, 8, N], F16, name=f"zall_{b}", tag="zall", bufs=2
                    )
                zall = cx["zall"]
                for nh in range(2):
                    ps = pso.tile([128, 512], F32, name=f"ps_z{b}{h}{nh}", tag="pso")
                    for kc in range(4):
                        if h < 2:
                            wslice = wztA[kc][:, h * 128:(h + 1) * 128]
                        else:
                            wslice = wztB[kc][:, (h - 2) * 128:(h - 1) * 128]
                        nc.tensor.matmul(
                            ps[:],
                            wslice,
                            cx["x"][kc][:, nh * 512:(nh + 1) * 512],
                            start=(kc == 0),
                            stop=(kc == 3),
                        )
                    nc.vector.tensor_scalar_add(
                        zall[:, h, nh * 512:(nh + 1) * 512], ps[:], bz_sb[:, h:h + 1]
                    )
                nc.sync.dma_start(uall[0:64, h, :], zall[64:128, h, :])

            def get_vpt(b):
                cx = ctx[b]
                if "vpt" not in cx:
                    cx["vpt"] = wpool.tile([128, 8, 520], BF16, name=f"vpt_{b}", tag="vpt")
                return cx["vpt"]

            def emit_vproj(b, c0, c1):
                # V^T padded projection (bf16 out), ones-column included via
                # the padded bias row; main/tail split keeps PSUM slots 1-bank.
                cx = ctx[b]
                vpt = get_vpt(b)
                for nc8 in range(c0, c1):
                    for (lo, hi) in ((0, 512), (512, 520)):
                        ps = pso.tile(
                            [128, hi - lo], F32, name=f"ps_v{b}{nc8}{lo}", tag="pso"
                        )
                        for kc in range(4):
                            nc.tensor.matmul(
                                ps[:],
                                cx["x"][kc][:, nc8 * 128:(nc8 + 1) * 128],
                                wvpt_sb[:, kc, lo:hi],
                                start=(kc == 0),
                                stop=False,
                            )
                        nc.tensor.matmul(
                            ps[:],
                            ones1[0:1, :],
                            bvp_sb[:, lo:hi],
                            start=False,
                            stop=True,
                        )
                        nc.vector.tensor_copy(vpt[:, nc8, lo:hi], ps[:])

            def get_osb(b):
                cx = ctx[b]
                if "osb" not in cx:
                    cx["osb"] = wpool.tile(
                        [65, 8, N], BF16, name=f"osb_{b}", tag="osb", bufs=2
                    )
                    cx["den_bf"] = wpool.tile(
                        [8, N], BF16, name=f"denbf_{b}", tag="denbf", bufs=2
                    )
                    cx["denf"] = wpool.tile(
                        [8, N], F32, name=f"denf_{b}", tag="denf", bufs=2
                    )
                    # rows are recip'd before all 8 heads land: keep unwritten
                    # rows finite (recip(1)=1) so masked-out lanes never NaN
                    nc.vector.memset(cx["den_bf"][:], 1.0)
                return cx["osb"], cx["den_bf"], cx["denf"]

            def emit_av_chunk(st, j):
                bp, hp, ptts, ops_a, ops_b = st
                pvpt = get_vpt(bp)
                for mh, ops in ((0, ops_a), (1, ops_b)):
                    nc.tensor.matmul(
                        ops[:],
                        pvpt[:, j, hp * 65:hp * 65 + 65],
                        ptts[j][:, mh * 512:(mh + 1) * 512],
                        start=(j == 0),
                        stop=(j == 7),
                    )

            def emit_av_evac(st, on_act=False):
                # evac normally on DVE (ACT-side evacs delay the next window's
                # first exps, stalling the lag-2 AV chain); the final head's
                # evac uses the by-then-idle Scalar engine.
                bp, hp, ptts, ops_a, ops_b = st
                osb, den_bf, denf = get_osb(bp)
                eng = nc.scalar.copy if on_act else nc.vector.tensor_copy
                eng(osb[:, hp, 0:512], ops_a[:])
                eng(osb[:, hp, 512:1024], ops_b[:])
                nc.sync.dma_start(den_bf[hp:hp + 1, :], osb[64:65, hp, :])

            def emit_head(b, h, carry, self_av=True, lag=3):
                # energy+exp for (b, h) with THIS head's AV interleaved at
                # lag-1 chunks (exp j-1 is done while E j streams).  The
                # previous head's final AV chunk + evac land at j=0 so its
                # normalize chain starts a full window earlier.
                cx = ctx[b]
                zall = cx["zall"]
                tts = []
                st = None
                for j in range(8):
                    eps = pse.tile([128, N], F32, name=f"ps_e{b}{h}{j}", tag="pse")
                    for ih in range(2):
                        nc.tensor.matmul(
                            eps[:, ih * 512:(ih + 1) * 512],
                            zall[:, h, j * 128:(j + 1) * 128],
                            uall[:, h, ih * 512:(ih + 1) * 512],
                            start=True,
                            stop=True,
                        )
                    if j in (2, 3, 4) and carry is not None:
                        emit_av_chunk(carry, j + 3)
                    if j == 5 and carry is not None:
                        emit_av_evac(carry)
                    if j > lag - 1 and self_av:
                        if st is None:
                            oa = pso.tile([65, 512], F32, name=f"ps_oa{b}{h}", tag="pso")
                            ob = pso.tile([65, 512], F32, name=f"ps_ob{b}{h}", tag="pso")
                            st = (b, h, tts, oa, ob)
                        emit_av_chunk(st, j - lag)
                    tt = wpool.tile([128, N], BF16, name=f"tt_{b}_{h}_{j}", tag="tt", bufs=12)
                    nc.scalar.activation(tt[:], eps[:], EXP, bias=zbias[:])
                    tts.append(tt)
                if not self_av:
                    return (b, h, tts, None, None)
                return st

            def emit_norm(b, h0, h1, use_pe=False):
                # normalize heads [h0, h1): reciprocal on just those den rows
                # (cost scales with free length, not partitions), hi+lo bf16
                # split, rank-1 broadcast, multiply, residual add, store.
                cx = ctx[b]
                osb, den_bf, denf = cx["osb"], cx["den_bf"], cx["denf"]
                # custom-DVE ops must start at partition 0: always process all
                # 8 rows (cost scales with free length, not partition count)
                nc.vector.tensor_copy(denf[:], den_bf[:])
                nc.vector.reciprocal_approx_fast(rinv[:], denf[:])
                nc.vector.tensor_copy(hi_t[:], rinv[:])
                for h in range(h0, h1):
                    nmul = wpool.tile([64, N], F16, name=f"nm_{b}_{h}", tag="nm", bufs=3)
                    if use_pe:
                        # low-latency PE rank-1 broadcast for tail-critical heads
                        for mh in range(2):
                            rb = pso.tile([64, 512], F32, name=f"ps_r{b}{h}{mh}", tag="pso")
                            nc.tensor.matmul(
                                rb[:], mask_sb[:, h * 64:(h + 1) * 64],
                                hi_t[:, mh * 512:(mh + 1) * 512],
                                start=True, stop=True,
                            )
                            nc.vector.tensor_mul(
                                nmul[:, mh * 512:(mh + 1) * 512],
                                osb[0:64, h, mh * 512:(mh + 1) * 512], rb[:],
                            )
                    else:
                        # SWDGE partition-broadcast (source must sit at
                        # partition 0, so hop the row there first) keeps the
                        # rank-1 replicate off the PE and makes the multiply
                        # a 16-bit 2x-mode DVE op
                        hst = wpool.tile([1, N], BF16, name=f"hst_{b}_{h}", tag="hst", bufs=2)
                        nc.sync.dma_start(hst[:], hi_t[h:h + 1, :])
                        rbv = wpool.tile([64, N], BF16, name=f"rbv_{b}_{h}", tag="rbv", bufs=2)
                        nc.gpsimd.partition_broadcast(rbv[:], hst[0:1, :])
                        nc.vector.tensor_mul(nmul[:], osb[0:64, h, :], rbv[:])
                    fin = wpool.tile([64, N], F16, name=f"fin_{b}_{h}", tag="fin", bufs=3)
                    if h % 2 == 0:
                        xres = cx["x"][h // 2][0:64, :]
                    else:
                        xres = cx["xodd"][:, h // 2, :]
                    nc.vector.tensor_add(fin[:], nmul[:], xres)
                    nc.sync.dma_start(out_d[b, h * 64:(h + 1) * 64, :], fin[:])

            # ---- prologue: batch 0 projections, head 0 early ----
            # head 0's AV must be EMITTED after the vproj writes (Tile RAW
            # deps look backward in emission order), so defer it here.
            emit_zproj_chunk(0, 0)
            emit_zproj_chunk(0, 1)
            emit_zproj_chunk(0, 2)
            b0, h0, tts0, _, _ = emit_head(0, 0, None, self_av=False)
            for h in range(3, 8):
                emit_zproj_chunk(0, h)
            emit_vproj(0, 0, 8)
            oa = pso.tile([65, 512], F32, name="ps_oa00", tag="pso")
            ob = pso.tile([65, 512], F32, name="ps_ob00", tag="pso")
            carry = (b0, h0, tts0, oa, ob)
            for j in range(5):
                emit_av_chunk(carry, j)

            # ---- steady state ----
            for b in range(BPC):
                for h in range(8):
                    if b == 0 and h == 0:
                        continue  # emitted in prologue
                    prev = carry
                    last = (b == BPC - 1 and h == 7)
                    carry = emit_head(b, h, carry, lag=1 if last else 3)
                    if b > 0 and h in (0, 1, 2, 3):
                        emit_zproj_chunk(b, h + 4)
                    if prev is not None and prev[1] in (2, 4, 6, 7):
                        emit_norm(prev[0], *{2: (0, 3), 4: (3, 5),
                                             6: (5, 7), 7: (7, 8)}[prev[1]],
                                  use_pe=(prev[0] == BPC - 1 and prev[1] == 6))
                    if b + 1 < BPC:
                        if h == 2:
                            ctx[b + 1] = {"x": prep_x(b + 1)}
                            prep_xodd(b + 1)
                        elif h in (4, 5, 6, 7):
                            emit_zproj_chunk(b + 1, h - 4)
                        if h == 6:
                            emit_vproj(b + 1, 0, 4)
                        elif h == 7:
                            emit_vproj(b + 1, 4, 8)

            # drain: last head's final AV chunks, then a half-pipelined
            # normalize chain (each 512-col half runs evac->den->recip->
            # broadcast->mul->add->store independently, halving tail latency)
            emit_av_chunk(carry, 7)
            bp, hp, ptts, ops_a, ops_b = carry
            osb, den_bf, denf = get_osb(bp)
            cxl = ctx[bp]
            nmul = wpool.tile([64, N], F16, name="nm_last", tag="nm", bufs=3)
            fin = wpool.tile([64, N], F16, name="fin_last", tag="fin", bufs=3)
            for mh, ops in ((0, ops_a), (1, ops_b)):
                sl = slice(mh * 512, (mh + 1) * 512)
                nc.scalar.copy(osb[:, hp, sl], ops[:])
                nc.sync.dma_start(den_bf[hp:hp + 1, sl], osb[64:65, hp, sl])
                nc.vector.tensor_copy(denf[:, sl], den_bf[:, sl])
                nc.vector.reciprocal_approx_fast(rinv[:, sl], denf[:, sl])
                nc.vector.tensor_copy(hi_t[:, sl], rinv[:, sl])
                rb = pso.tile([64, 512], F32, name=f"ps_rl{mh}", tag="pso")
                nc.tensor.matmul(
                    rb[:], mask_sb[:, hp * 64:(hp + 1) * 64], hi_t[:, sl],
                    start=True, stop=True,
                )
                nc.vector.tensor_mul(nmul[:, sl], osb[0:64, hp, sl], rb[:])
                nc.vector.tensor_add(
                    fin[:, sl], nmul[:, sl], cxl["xodd"][:, hp // 2, sl]
                )
                nc.sync.dma_start(out_d[bp, hp * 64:(hp + 1) * 64, sl], fin[:, sl])

    nc.compile()
    return nc


def _prep_consts(Wq, bq, Wk, bk, Wv, bv, rel_h, rel_w):
    # interleaved Z weights: chunk h rows 0-63 = Wk head h, rows 64-127 = Wq
    Wz = np.zeros((1024, 512), np.float32)
    bzv = np.zeros((1024,), np.float32)
    for h in range(HEAD):
        Wz[h * 128:h * 128 + 64] = Wk[h * 64:(h + 1) * 64]
        Wz[h * 128 + 64:h * 128 + 128] = Wq[h * 64:(h + 1) * 64]
        bzv[h * 128:h * 128 + 64] = bk[h * 64:(h + 1) * 64]
        bzv[h * 128 + 64:h * 128 + 128] = bq[h * 64:(h + 1) * 64]
    wzt = np.ascontiguousarray(Wz.T).reshape(4, 128, 1024).astype(np.float16)
    wzta = np.ascontiguousarray(wzt[:, :, 0:256])
    wztb = np.ascontiguousarray(wzt[:, :, 256:1024])
    bz = np.ascontiguousarray(bzv.reshape(8, 128).T).astype(np.float32)

    wvpt = np.zeros((512, 520), np.float32)
    bvp = np.zeros((1, 520), np.float32)
    for h in range(HEAD):
        wvpt[:, h * 65:h * 65 + 64] = Wv[h * 64:(h + 1) * 64, :].T
        bvp[0, h * 65:h * 65 + 64] = bv[h * 64:(h + 1) * 64]
        bvp[0, h * 65 + 64] = 1.0

    mask = np.zeros((8, 512), np.float32)
    for h in range(HEAD):
        mask[h, h * 64:(h + 1) * 64] = 1.0

    pos = (rel_h + rel_w).reshape(HEAD, D, N).astype(np.float16)
    return {
        "wzta": wzta,
        "wztb": wztb,
        "bz": bz,
        "wvpt": wvpt.reshape(4, 128, 520).astype(np.float16),
        "bvp": bvp.astype(np.float16),
        "mask": mask.astype(ml_dtypes.bfloat16),
        "pos": pos,
    }


_CACHE = {}


def build_in_maps(x, Wq, bq, Wk, bk, Wv, bv, rel_h, rel_w):
    x = np.asarray(x, np.float32)
    consts = _prep_consts(
        *[np.asarray(a, np.float32) for a in (Wq, bq, Wk, bk, Wv, bv, rel_h, rel_w)]
    )
    xh = x.reshape(B, C, N).astype(np.float16)
    in_maps = []
    for c in range(N_CORES):
        m = dict(consts)
        m["xh"] = np.ascontiguousarray(xh[c * BPC:(c + 1) * BPC])
        in_maps.append(m)
    return in_maps


def kernel(x, Wq, bq, Wk, bk, Wv, bv, rel_h, rel_w, reg_qk, reg_v):
    # reg_qk / reg_v are computed-then-dropped by the reference -> unused.
    in_maps = build_in_maps(x, Wq, bq, Wk, bk, Wv, bv, rel_h, rel_w)

    if "nc" not in _CACHE:
        _CACHE["nc"] = build_bass()
    res = run_bass_kernel_spmd(_CACHE["nc"], in_maps, list(range(N_CORES)))
    outs = [np.asarray(r["out"]).astype(np.float32) for r in res.results]
    return np.concatenate(outs, axis=0).reshape(B, C, WD, HD)


if __name__ == "__main__":
    nc = build_bass()
    print("built ok")
